# revision 1
# baseline (speedup 1.0000x reference)
"""Trainium2 Bass kernel for LocallyDirected1D (gnn_message_passing).

Computation: out[b, j] = sum_{e in [o[j], o[j+1])} x[b, e] * k[e]  (+ bias[j])
where o = CSR offsets of the sorted mask_col, mask_row == arange(E).

Strategy:
  * Data-parallel over batch: core c handles batch rows [8c, 8c+8).
  * Host builds a padded-ELL layout: output columns sorted by segment
    length, grouped into superblocks of 1024 columns (8 groups x 128
    partitions) padded to the superblock max length P.  Padding slots of
    the weight array are zero, so the gathered x can hold garbage there.
    The x slab and the k slab of a superblock are packed into ONE
    contiguous HBM region so a single DMA (= a single semaphore wait)
    brings in both.
  * Device streams the dense slabs: y = x*k on VectorE, segment sums via
    tensor_reduce over the innermost (padded-length) axis, results DMAed
    back and un-permuted on host.
"""

import numpy as np

import concourse.bass as bass
import concourse.mybir as mybir
from concourse.tile import TileContext
from concourse.bass_utils import run_bass_kernel_spmd

B = 64
E = 1_000_000
NOUT = 20_000
NCORES = 8
BPC = B // NCORES          # batch rows per core
CPB = 128                  # columns per block (partition dim)
G = 8                      # blocks per superblock
SBC = CPB * G              # 1024 columns per superblock
NSB = (NOUT + SBC - 1) // SBC   # 20 superblocks
NPAD = NSB * SBC - NOUT    # dummy (zero-length) columns, placed first
ROWW = BPC * G + G         # free elems per partition per unit P (x then k)

F32 = mybir.dt.float32


def _plan(mask_col: np.ndarray):
    """CSR offsets -> length-sorted padded-ELL plan."""
    o = np.searchsorted(mask_col, np.arange(NOUT + 1)).astype(np.int64)
    lens = np.diff(o).astype(np.int64)
    perm = np.argsort(lens, kind="stable").astype(np.int64)
    lens_s = np.concatenate([np.zeros(NPAD, np.int64), lens[perm]])
    starts_s = np.concatenate([np.zeros(NPAD, np.int64), o[:-1][perm]])
    P = lens_s.reshape(NSB, SBC).max(axis=1)
    P = np.maximum(P, 1).astype(np.int64)
    return perm, lens_s, starts_s, P


def _build_program(P, gp_mod=2, gp_keep=1):
    """gp_mod/gp_keep: superblocks with sb % gp_mod < gp_keep run their
    multiply on GPSIMD (~2x slower per element than VectorE but fully
    concurrent); the rest multiply on VectorE.  All reduces are VectorE
    (the only engine with free-axis tensor_reduce)."""
    nc = bass.Bass()
    off = np.concatenate([[0], np.cumsum(128 * ROWW * P)]).astype(np.int64)
    xk_d = nc.dram_tensor("xkell", [int(off[-1])], F32, kind="ExternalInput")
    # one output tensor per superblock: a single shared output tensor makes
    # Tile serialize the store DMAs (tensor-granularity WAW), which puts a
    # second sync-wait on each store — walrus allows only one per instruction
    o_ds = [
        nc.dram_tensor(f"oseg{sb:02d}", [128 * BPC * G], F32,
                       kind="ExternalOutput")
        for sb in range(NSB)
    ]

    with TileContext(nc) as tc:
        with (
            tc.tile_pool(name="xp", bufs=4) as xp,
            tc.tile_pool(name="op", bufs=NSB) as op_,
        ):
            for sb in range(NSB):
                Ps = int(P[sb])
                QF = G * Ps            # free elems per batch row (and k width)
                XF = BPC * QF          # x portion width
                t = xp.tile([128, ROWW * Ps], F32, tag="x")
                nc.sync.dma_start(
                    t[:],
                    xk_d[int(off[sb]):int(off[sb + 1])].rearrange(
                        "(j f) -> j f", j=128),
                )
                eng = nc.gpsimd if (sb % gp_mod) < gp_keep else nc.vector
                xv = t[:, 0:XF].rearrange("j (b q) -> j b q", b=BPC)
                kv = (t[:, XF:XF + QF].unsqueeze(1)
                      .broadcast_to([128, BPC, QF]))
                eng.tensor_tensor(xv, xv, kv, mybir.AluOpType.mult)
                ot = op_.tile([128, BPC * G], F32, tag="o")
                nc.vector.tensor_reduce(
                    ot[:],
                    t[:, 0:XF].rearrange("j (q p) -> j q p", p=Ps),
                    axis=mybir.AxisListType.X,
                    op=mybir.AluOpType.add,
                )
                nc.sync.dma_start(
                    o_ds[sb][:].rearrange("(j f) -> j f", j=128),
                    ot[:],
                )
    return nc, off


def _split_multi_waits(nc):
    """walrus allows at most one sync-wait per engine instruction; hoist
    extra waits into standalone EventSemaphore sequencer instructions
    placed immediately before (same engine => same stream order)."""
    from bass_rust import SyncInfo
    n = 0
    for f in nc.m.functions:
        for blk in f.blocks:
            new = []
            for inst in blk.instructions:
                si = inst.sync_info
                if si is not None and len(si.on_wait) > 1:
                    for w in si.on_wait[:-1]:
                        n += 1
                        new.append(mybir.InstEventSemaphore(
                            name=f"evw-{n}", engine=inst.engine,
                            sync_info=SyncInfo(on_wait=[w], on_update=[]),
                        ))
                    inst.sync_info = SyncInfo(on_wait=[si.on_wait[-1]],
                                              on_update=list(si.on_update))
                new.append(inst)
            try:
                blk.instructions = new
            except Exception:
                blk.instructions[:] = new
    return n


def _host_pack(x2, kflat, lens_s, starts_s, P, off):
    """Gather x / kernel into the packed padded-ELL slabs, per core."""
    xkell = [np.empty(int(off[-1]), np.float32) for _ in range(NCORES)]
    for sb in range(NSB):
        Ps = int(P[sb])
        st = starts_s[sb * SBC:(sb + 1) * SBC].reshape(G, CPB)
        ln = lens_s[sb * SBC:(sb + 1) * SBC].reshape(G, CPB)
        pr = np.arange(Ps)
        msk = pr < ln[..., None]                        # [G,128,Ps]
        idx = np.where(msk, st[..., None] + pr, 0)
        kslab = ((kflat[idx] * msk).astype(np.float32)
                 .transpose(1, 0, 2).reshape(128, G * Ps))
        gx = x2[:, idx]                                 # [B,G,128,Ps]
        for c in range(NCORES):
            xs = (gx[c * BPC:(c + 1) * BPC]
                  .transpose(2, 0, 1, 3).reshape(128, BPC * G * Ps))
            xkell[c][int(off[sb]):int(off[sb + 1])] = (
                np.concatenate([xs, kslab], axis=1).ravel())
    return xkell


def kernel(x, kernel, bias, mask_row, mask_col, _trace=False):
    x = np.asarray(x, np.float32)
    kflat = np.asarray(kernel, np.float32).reshape(E)
    bias = np.asarray(bias, np.float32)
    mask_col = np.asarray(mask_col)
    x2 = np.ascontiguousarray(x.reshape(B, E))

    perm, lens_s, starts_s, P = _plan(mask_col)
    nc, off = _build_program(P)
    _split_multi_waits(nc)
    xkell = _host_pack(x2, kflat, lens_s, starts_s, P, off)

    in_maps = [{"xkell": xkell[c]} for c in range(NCORES)]
    res = run_bass_kernel_spmd(
        nc, in_maps, core_ids=list(range(NCORES)), trace=_trace)

    out_full = np.zeros((B, NOUT), np.float32)
    for c in range(NCORES):
        arr = np.stack([res.results[c][f"oseg{sb:02d}"]
                        for sb in range(NSB)]).reshape(NSB, 128, BPC, G)
        vals = arr.transpose(2, 0, 3, 1).reshape(BPC, NSB * SBC)[:, NPAD:]
        out_full[c * BPC:(c + 1) * BPC, perm] = vals

    out = out_full[:, :, None] + bias[None, :, :]
    if _trace:
        return out, res
    return out



# revision 16
# speedup vs baseline: 2.0372x; 2.0372x over previous
"""Trainium2 Bass kernel for LocallyDirected1D (gnn_message_passing).

Computation: out[b, j] = sum_{e: mask_col[e]==j} x[b, mask_row[e]] * k[e]
(+ bias[j]); mask_col is sorted, so segments are contiguous edge ranges.

Strategy (v8 — PSUM accumulation chains, full-row packing, bf16):
  * Data-parallel over batch: core c handles batch rows [8c, 8c+8).
  * Segments (output cols) sorted by length; 64 consecutive sorted
    segments form a "run" (slot height h = max length, tight after
    sorting).  32 consecutive runs form a "bin" = one 32-partition
    column-group of a PSUM bank.  A bin's runs are laid out as one tall
    column of Sum(h) rows, cut into 128-row matmuls that ACCUMULATE into
    the same PSUM region (start on the first, stop on the last) — runs
    straddle matmul boundaries freely, so every matmul uses all 128
    contraction rows (~98% fill).
  * Slab per matmul: [128 rows, 512 x | 64 k | 32 S] bf16, where S is
    the slotted-ones stationary.  VectorE multiplies x *= k in place
    (2x DVE mode), TensorE reduces along partitions, 4 bins fill a PSUM
    bank [128, 512] that ScalarE drains once and one DMA stores.
  * All tiles are full 128 partitions and rows are 256B-aligned (the
    DMA AP splitter only sprays descriptors across all 16 SDMA engines
    for full, aligned tiles).  Input DMAs ride the SP HWDGE ring;
    output DMAs ride the ACT ring so stores never stall loads.
All arithmetic (multiply + reduction) happens on device; the host does
layout only (gather/pad/cast) and the final permutation.
"""

import numpy as np
import ml_dtypes

import concourse.bass as bass
import concourse.mybir as mybir
from concourse.tile import TileContext
from concourse.bass_utils import run_bass_kernel_spmd

B = 64
E = 1_000_000
NOUT = 20_000
NCORES = 8
BPC = B // NCORES          # batch rows per core
QPACK = 64                 # segments per run (= packs per matmul)
NMOV = BPC * QPACK         # moving columns per matmul (= bank width)
RPB = 32                   # runs per bin (= PSUM partitions per bin)
GW = 608                   # slab cols per matmul: 512 x + 64 k + 32 S

F32 = mybir.dt.float32
BF16 = mybir.dt.bfloat16
NPBF = ml_dtypes.bfloat16


def _plan(lens):
    """runs of 64 length-sorted segs -> bins of 32 runs -> 128-row matmuls."""
    nz = np.flatnonzero(lens > 0)
    order = nz[np.argsort(lens[nz], kind="stable")]
    assert order.size == 0 or lens[order[-1]] <= 128, "segment longer than 128"
    nrun = (order.size + QPACK - 1) // QPACK
    segs = np.full(nrun * QPACK, -1, np.int64)
    segs[:order.size] = order
    segs = segs.reshape(nrun, QPACK)
    h = np.maximum(lens[np.maximum(segs, 0)].max(axis=1), 1)
    h[segs.max(axis=1) < 0] = 1
    bins = []
    for b0 in range(0, nrun, RPB):
        rs = list(range(b0, min(b0 + RPB, nrun)))
        rows = int(h[rs].sum())
        n_mm = (rows + 127) // 128
        bins.append({"runs": rs, "rows": rows, "n_mm": n_mm,
                     "rho": np.concatenate([[0], np.cumsum(h[rs])])})
    return segs, h, bins


def _build_program(bins):
    nc = bass.Bass()
    total = sum(b["nelem"] for b in bins)
    xk_d = nc.dram_tensor("xkell", [total], BF16, kind="ExternalInput")
    n_pt = (len(bins) + 3) // 4
    o_ds = [nc.dram_tensor(f"oc{p}", [128 * NMOV], F32,
                           kind="ExternalOutput") for p in range(n_pt)]

    with TileContext(nc) as tc:
        with (
            tc.tile_pool(name="xp", bufs=4) as xp,
            tc.psum_pool(name="pp", bufs=2) as pp,
            tc.tile_pool(name="dp", bufs=2) as dp,
        ):
            pt = None
            for bi, bn in enumerate(bins):
                n_mm, W = bn["n_mm"], bn["W"]
                j = bi % 4
                t = xp.tile([128, W], BF16, tag="x", name=f"xt{bi}")
                nc.sync.dma_start(
                    t[:],
                    xk_d[bn["off"]:bn["off"] + bn["nelem"]].rearrange(
                        "(p w) -> p w", p=128),
                )
                xk = t[:, 0:GW * n_mm].rearrange("p (g c) -> p g c", g=n_mm)
                xa = xk[:, :, 0:512].rearrange("p g (b q) -> p g b q", b=BPC)
                kv = (xk[:, :, 512:576].unsqueeze(2)
                      .broadcast_to([128, n_mm, BPC, QPACK]))
                nc.vector.tensor_tensor(xa, xa, kv, mybir.AluOpType.mult)

                if j == 0:
                    pt = pp.tile([128, NMOV], F32, tag="ps", name=f"pt{bi}")
                for i in range(n_mm):
                    off = GW * i
                    nc.tensor.matmul(
                        pt[32 * j:32 * j + 32, :],
                        lhsT=t[:, off + 576:off + 608],
                        rhs=t[:, off:off + 512],
                        start=(i == 0), stop=(i == n_mm - 1),
                        tile_position=(0, 32 * j),
                    )
                if j == 3 or bi == len(bins) - 1:
                    p = bi // 4
                    dt = dp.tile([128, NMOV], F32, tag="d", name=f"dt{p}")
                    nc.scalar.copy(dt[:], pt[:])
                    nc.scalar.dma_start(
                        o_ds[p][:].rearrange("(r f) -> r f", r=128), dt[:])
    return nc


def _split_multi_waits(nc):
    """walrus allows at most one sync-wait per engine instruction; hoist
    extra waits into standalone EventSemaphore sequencer instructions
    placed immediately before (same engine => same stream order)."""
    from bass_rust import SyncInfo
    n = 0
    for f in nc.m.functions:
        for blk in f.blocks:
            new = []
            for inst in blk.instructions:
                si = inst.sync_info
                if si is not None and len(si.on_wait) > 1:
                    for w in si.on_wait[:-1]:
                        n += 1
                        new.append(mybir.InstEventSemaphore(
                            name=f"evw-{n}", engine=inst.engine,
                            sync_info=SyncInfo(on_wait=[w], on_update=[]),
                        ))
                    inst.sync_info = SyncInfo(on_wait=[si.on_wait[-1]],
                                              on_update=list(si.on_update))
                new.append(inst)
            try:
                blk.instructions = new
            except Exception:
                blk.instructions[:] = new
    return n


def kernel(x, kernel, bias, mask_row, mask_col, _trace=False, _cores=None):
    x = np.asarray(x, np.float32)
    kflat = np.asarray(kernel, np.float32).reshape(E)
    bias = np.asarray(bias, np.float32)
    mask_row = np.asarray(mask_row, np.int64)
    mask_col = np.asarray(mask_col)
    x2 = np.ascontiguousarray(x.reshape(B, E))

    o = np.searchsorted(mask_col, np.arange(NOUT + 1)).astype(np.int64)
    lens = np.diff(o)
    segs, h, bins = _plan(lens)

    off = 0
    for bn in bins:
        W = (GW * bn["n_mm"] + 127) // 128 * 128
        bn["W"] = W
        bn["off"] = off
        bn["nelem"] = 128 * W
        off += 128 * W

    x_bf = x2.astype(NPBF)
    k_bf = kflat.astype(NPBF)
    seg_start = o[:-1]
    seg_len = lens

    core_slabs = [np.zeros(off, NPBF) for _ in range(NCORES)]
    for bn in bins:
        n_mm, W = bn["n_mm"], bn["W"]
        rho = bn["rho"]
        gmap = np.full((128, n_mm * QPACK), -1, np.int64)   # [row, (mm, q)]
        s_all = np.zeros((128, n_mm, 32), NPBF)
        for tl, r in enumerate(bn["runs"]):
            sr = segs[r]                                     # [QPACK]
            valid = sr >= 0
            st = np.where(valid, seg_start[np.maximum(sr, 0)], 0)
            ln = np.where(valid, seg_len[np.maximum(sr, 0)], 0)
            g0, g1 = int(rho[tl]), int(rho[tl + 1])          # bin-global rows
            for i in range(g0 // 128, (g1 - 1) // 128 + 1):
                a, bnd = max(g0, i * 128), min(g1, (i + 1) * 128)
                u = np.arange(a - g0, bnd - g0)              # edge offsets
                msk = u[:, None] < ln[None, :]
                eid = np.where(msk, st[None, :] + u[:, None], -1)
                gmap[a - i * 128:bnd - i * 128,
                     i * QPACK:(i + 1) * QPACK] = eid
                s_all[a - i * 128:bnd - i * 128, i, tl] = 1.0
        gclip = np.maximum(gmap, 0)
        kvals = np.where(gmap >= 0, k_bf[gclip], NPBF(0))    # [128, n_mm*64]
        xsrc = mask_row[gclip]
        gx = x_bf[:, xsrc.reshape(-1)].reshape(B, 128, n_mm, QPACK)
        for c in range(NCORES):
            A = np.zeros((128, W), NPBF)
            av = A[:, 0:GW * n_mm].reshape(128, n_mm, GW)
            part = gx[c * BPC:(c + 1) * BPC]                 # [8,128,n_mm,64]
            av[:, :, 0:512] = (part.transpose(1, 2, 0, 3)
                               .reshape(128, n_mm, 512))
            av[:, :, 512:576] = kvals.reshape(128, n_mm, QPACK)
            av[:, :, 576:608] = s_all
            core_slabs[c][bn["off"]:bn["off"] + bn["nelem"]] = A.reshape(-1)

    global _last_plan
    _last_plan = (segs, h, bins)
    nc = _build_program(bins)
    _split_multi_waits(nc)

    cores = list(range(NCORES)) if _cores is None else _cores
    in_maps = [{"xkell": core_slabs[c]} for c in cores]
    res = run_bass_kernel_spmd(nc, in_maps, core_ids=cores, trace=_trace)

    # unscramble: seg (bin bi, run tl, pack q) -> oc{bi//4} flat index
    # layout [partition = 32*(bi%4)+tl, b, q]
    seg_ids, seg_flat = [], []
    for bi, bn in enumerate(bins):
        for tl, r in enumerate(bn["runs"]):
            sr = segs[r]
            qq = np.flatnonzero(sr >= 0)
            seg_ids.append(sr[qq])
            base = (bi // 4) * 128 * NMOV
            part = 32 * (bi % 4) + tl
            seg_flat.append(base + (part * BPC) * QPACK + qq)
    seg_ids = np.concatenate(seg_ids)
    seg_flat = np.concatenate(seg_flat)

    n_pt = (len(bins) + 3) // 4
    out_full = np.zeros((B, NOUT), np.float32)
    for idx, c in enumerate(cores):
        F = np.concatenate([np.asarray(res.results[idx][f"oc{p}"],
                                       np.float32).reshape(-1)
                            for p in range(n_pt)])
        rows = F[seg_flat[None, :] + (np.arange(BPC) * QPACK)[:, None]]
        out_full[c * BPC:(c + 1) * BPC, seg_ids] = rows

    out = out_full[:, :, None] + bias[None, :, :]
    if _trace:
        return out, res
    return out


# revision 21
# speedup vs baseline: 2.0857x; 1.0238x over previous
"""Trainium2 Bass kernel for LocallyDirected1D (gnn_message_passing).

Computation: out[b, j] = sum_{e: mask_col[e]==j} x[b, mask_row[e]] * k[e]
(+ bias[j]); mask_col is sorted, so segments are contiguous edge ranges.

Strategy (v8 — PSUM accumulation chains, full-row packing, bf16):
  * Data-parallel over batch: core c handles batch rows [8c, 8c+8).
  * Segments (output cols) sorted by length; 64 consecutive sorted
    segments form a "run" (slot height h = max length, tight after
    sorting).  32 consecutive runs form a "bin" = one 32-partition
    column-group of a PSUM bank.  A bin's runs are laid out as one tall
    column of Sum(h) rows, cut into 128-row matmuls that ACCUMULATE into
    the same PSUM region (start on the first, stop on the last) — runs
    straddle matmul boundaries freely, so every matmul uses all 128
    contraction rows (~98% fill).
  * Slab per matmul: [128 rows, 512 x | 64 k | 32 S] bf16, where S is
    the slotted-ones stationary.  VectorE multiplies x *= k in place
    (2x DVE mode), TensorE reduces along partitions, 4 bins fill a PSUM
    bank [128, 512] that ScalarE drains once and one DMA stores.
  * All tiles are full 128 partitions and rows are 256B-aligned (the
    DMA AP splitter only sprays descriptors across all 16 SDMA engines
    for full, aligned tiles).  Input DMAs ride the SP HWDGE ring;
    output DMAs ride the ACT ring so stores never stall loads.
All arithmetic (multiply + reduction) happens on device; the host does
layout only (gather/pad/cast) and the final permutation.
"""

import numpy as np
import ml_dtypes

import concourse.bass as bass
import concourse.mybir as mybir
from concourse.tile import TileContext
from concourse.bass_utils import run_bass_kernel_spmd

B = 64
E = 1_000_000
NOUT = 20_000
NCORES = 8
BPC = B // NCORES          # batch rows per core
QPACK = 64                 # segments per run (= packs per matmul)
NMOV = BPC * QPACK         # moving columns per matmul (= bank width)
RPB = 32                   # runs per bin (= PSUM partitions per bin)
GW = 608                   # slab cols per matmul: 512 x + 64 k + 32 S

F32 = mybir.dt.float32
BF16 = mybir.dt.bfloat16
NPBF = ml_dtypes.bfloat16


def _plan(lens):
    """runs of 64 length-sorted segs -> bins of 32 runs -> 128-row matmuls."""
    nz = np.flatnonzero(lens > 0)
    order = nz[np.argsort(lens[nz], kind="stable")]
    assert order.size == 0 or lens[order[-1]] <= 128, "segment longer than 128"
    nrun = (order.size + QPACK - 1) // QPACK
    segs = np.full(nrun * QPACK, -1, np.int64)
    segs[:order.size] = order
    segs = segs.reshape(nrun, QPACK)
    h = np.maximum(lens[np.maximum(segs, 0)].max(axis=1), 1)
    h[segs.max(axis=1) < 0] = 1
    # bin sizes: 32 runs each, but split the remainder into shrinking
    # trailing bins so the serial tail (last DMA -> TT -> MM -> drain)
    # is short
    sizes = []
    rem = nrun
    while rem > RPB + 8:
        sizes.append(RPB)
        rem -= RPB
    while rem > 0:
        s = max(1, min(rem, (rem + 1) // 2)) if rem > 4 else rem
        sizes.append(s)
        rem -= s
    bins = []
    b0 = 0
    for s in sizes:
        rs = list(range(b0, b0 + s))
        b0 += s
        rows = int(h[rs].sum())
        n_mm = (rows + 127) // 128
        bins.append({"runs": rs, "rows": rows, "n_mm": n_mm,
                     "rho": np.concatenate([[0], np.cumsum(h[rs])])})
    return segs, h, bins


def _build_program(bins):
    nc = bass.Bass()
    total = sum(s["nelem"] for b in bins for s in b["subs"])
    xk_d = nc.dram_tensor("xkell", [total], BF16, kind="ExternalInput")
    n_pt = (len(bins) + 3) // 4
    o_ds = [nc.dram_tensor(f"oc{p}", [128 * NMOV], F32,
                           kind="ExternalOutput") for p in range(n_pt)]

    with TileContext(nc) as tc:
        with (
            tc.tile_pool(name="xp", bufs=6) as xp,
            tc.psum_pool(name="pp", bufs=2) as pp,
            tc.tile_pool(name="dp", bufs=2) as dp,
        ):
            pt = None
            for bi, bn in enumerate(bins):
                n_mm = bn["n_mm"]
                j = bi % 4
                if j == 0:
                    pt = pp.tile([128, NMOV], F32, tag="ps", name=f"pt{bi}")
                for si, sub in enumerate(bn["subs"]):
                    nsub = sub["mm1"] - sub["mm0"]
                    W = sub["W"]
                    t = xp.tile([128, W], BF16, tag="x", name=f"xt{bi}_{si}")
                    nc.sync.dma_start(
                        t[:],
                        xk_d[sub["off"]:sub["off"] + sub["nelem"]].rearrange(
                            "(p w) -> p w", p=128),
                    )
                    xk = t[:, 0:GW * nsub].rearrange("p (g c) -> p g c",
                                                     g=nsub)
                    xa = xk[:, :, 0:512].rearrange("p g (b q) -> p g b q",
                                                   b=BPC)
                    kv = (xk[:, :, 512:576].unsqueeze(2)
                          .broadcast_to([128, nsub, BPC, QPACK]))
                    nc.vector.tensor_tensor(xa, xa, kv, mybir.AluOpType.mult)
                    for il in range(nsub):
                        i = sub["mm0"] + il
                        off = GW * il
                        nc.tensor.matmul(
                            pt[32 * j:32 * j + 32, :],
                            lhsT=t[:, off + 576:off + 608],
                            rhs=t[:, off:off + 512],
                            start=(i == 0), stop=(i == n_mm - 1),
                            tile_position=(0, 32 * j),
                        )
                if j == 3 or bi == len(bins) - 1:
                    p = bi // 4
                    dt = dp.tile([128, NMOV], F32, tag="d", name=f"dt{p}")
                    nc.scalar.copy(dt[:], pt[:])
                    nc.scalar.dma_start(
                        o_ds[p][:].rearrange("(r f) -> r f", r=128), dt[:])
    return nc


def _split_multi_waits(nc):
    """walrus allows at most one sync-wait per engine instruction; hoist
    extra waits into standalone EventSemaphore sequencer instructions
    placed immediately before (same engine => same stream order)."""
    from bass_rust import SyncInfo
    n = 0
    for f in nc.m.functions:
        for blk in f.blocks:
            new = []
            for inst in blk.instructions:
                si = inst.sync_info
                if si is not None and len(si.on_wait) > 1:
                    for w in si.on_wait[:-1]:
                        n += 1
                        new.append(mybir.InstEventSemaphore(
                            name=f"evw-{n}", engine=inst.engine,
                            sync_info=SyncInfo(on_wait=[w], on_update=[]),
                        ))
                    inst.sync_info = SyncInfo(on_wait=[si.on_wait[-1]],
                                              on_update=list(si.on_update))
                new.append(inst)
            try:
                blk.instructions = new
            except Exception:
                blk.instructions[:] = new
    return n


def kernel(x, kernel, bias, mask_row, mask_col, _trace=False, _cores=None):
    x = np.asarray(x, np.float32)
    kflat = np.asarray(kernel, np.float32).reshape(E)
    bias = np.asarray(bias, np.float32)
    mask_row = np.asarray(mask_row, np.int64)
    mask_col = np.asarray(mask_col)
    x2 = np.ascontiguousarray(x.reshape(B, E))

    o = np.searchsorted(mask_col, np.arange(NOUT + 1)).astype(np.int64)
    lens = np.diff(o)
    segs, h, bins = _plan(lens)

    off = 0
    for bn in bins:
        n_mm = bn["n_mm"]
        cuts_mm = [0, (n_mm + 1) // 2, n_mm] if n_mm >= 6 else [0, n_mm]
        bn["subs"] = []
        for mm0, mm1 in zip(cuts_mm[:-1], cuts_mm[1:]):
            W = (GW * (mm1 - mm0) + 127) // 128 * 128
            bn["subs"].append({"mm0": mm0, "mm1": mm1, "W": W,
                               "off": off, "nelem": 128 * W})
            off += 128 * W

    x_bf = x2.astype(NPBF)
    k_bf = kflat.astype(NPBF)
    seg_start = o[:-1]
    seg_len = lens

    core_slabs = [np.zeros(off, NPBF) for _ in range(NCORES)]
    for bn in bins:
        n_mm = bn["n_mm"]
        rho = bn["rho"]
        gmap = np.full((128, n_mm * QPACK), -1, np.int64)   # [row, (mm, q)]
        s_all = np.zeros((128, n_mm, 32), NPBF)
        for tl, r in enumerate(bn["runs"]):
            sr = segs[r]                                     # [QPACK]
            valid = sr >= 0
            st = np.where(valid, seg_start[np.maximum(sr, 0)], 0)
            ln = np.where(valid, seg_len[np.maximum(sr, 0)], 0)
            g0, g1 = int(rho[tl]), int(rho[tl + 1])          # bin-global rows
            for i in range(g0 // 128, (g1 - 1) // 128 + 1):
                a, bnd = max(g0, i * 128), min(g1, (i + 1) * 128)
                u = np.arange(a - g0, bnd - g0)              # edge offsets
                msk = u[:, None] < ln[None, :]
                eid = np.where(msk, st[None, :] + u[:, None], -1)
                gmap[a - i * 128:bnd - i * 128,
                     i * QPACK:(i + 1) * QPACK] = eid
                s_all[a - i * 128:bnd - i * 128, i, tl] = 1.0
        gclip = np.maximum(gmap, 0)
        kvals = np.where(gmap >= 0, k_bf[gclip], NPBF(0))    # [128, n_mm*64]
        kvals = kvals.reshape(128, n_mm, QPACK)
        xsrc = mask_row[gclip]
        gx = x_bf[:, xsrc.reshape(-1)].reshape(B, 128, n_mm, QPACK)
        for c in range(NCORES):
            part = gx[c * BPC:(c + 1) * BPC]                 # [8,128,n_mm,64]
            xb = part.transpose(1, 2, 0, 3).reshape(128, n_mm, 512)
            for sub in bn["subs"]:
                m0, m1, W = sub["mm0"], sub["mm1"], sub["W"]
                A = np.zeros((128, W), NPBF)
                av = A[:, 0:GW * (m1 - m0)].reshape(128, m1 - m0, GW)
                av[:, :, 0:512] = xb[:, m0:m1]
                av[:, :, 512:576] = kvals[:, m0:m1]
                av[:, :, 576:608] = s_all[:, m0:m1]
                core_slabs[c][sub["off"]:sub["off"] + sub["nelem"]] = (
                    A.reshape(-1))

    global _last_plan
    _last_plan = (segs, h, bins)
    nc = _build_program(bins)
    _split_multi_waits(nc)

    cores = list(range(NCORES)) if _cores is None else _cores
    in_maps = [{"xkell": core_slabs[c]} for c in cores]
    res = run_bass_kernel_spmd(nc, in_maps, core_ids=cores, trace=_trace)

    # unscramble: seg (bin bi, run tl, pack q) -> oc{bi//4} flat index
    # layout [partition = 32*(bi%4)+tl, b, q]
    seg_ids, seg_flat = [], []
    for bi, bn in enumerate(bins):
        for tl, r in enumerate(bn["runs"]):
            sr = segs[r]
            qq = np.flatnonzero(sr >= 0)
            seg_ids.append(sr[qq])
            base = (bi // 4) * 128 * NMOV
            part = 32 * (bi % 4) + tl
            seg_flat.append(base + (part * BPC) * QPACK + qq)
    seg_ids = np.concatenate(seg_ids)
    seg_flat = np.concatenate(seg_flat)

    n_pt = (len(bins) + 3) // 4
    out_full = np.zeros((B, NOUT), np.float32)
    for idx, c in enumerate(cores):
        F = np.concatenate([np.asarray(res.results[idx][f"oc{p}"],
                                       np.float32).reshape(-1)
                            for p in range(n_pt)])
        rows = F[seg_flat[None, :] + (np.arange(BPC) * QPACK)[:, None]]
        out_full[c * BPC:(c + 1) * BPC, seg_ids] = rows

    out = out_full[:, :, None] + bias[None, :, :]
    if _trace:
        return out, res
    return out
